# revision 30
# baseline (speedup 1.0000x reference)
"""Trainium2 Bass kernel for nn_Encoders (2-layer shared-weight transformer encoder).

Sharding (v3): 8 cores; pair (2b, 2b+1) handles batch b.  Within a pair the
split is along the attention *output* token axis j (the reference's unusual
attention contracts over queries i: out[j,d] = sum_i attn[i,j] v[i,d]):

  - each core computes q and v for ALL tokens/heads (small duplication),
    k only for its own j-half,
  - E = exp((qk^T + mask*NEG)/8) for its own j columns, all heads,
  - attention output, out-projection, residual+LN1, full-DFF FFN,
    residual+LN2 for its own j-half only -- NO partial-sum collectives.

Cross-core data: the joint-softmax denominator Z (per head) is summed with a
32-byte AllReduce, and the layer output h is AllGathered (1MB) at the layer
boundary; the final layer outputs each core's own half directly.

Everything stays in transposed layout [feature, token]; LN stats via all-ones
matmul (broadcast sums).  Matmul inputs are float32r (full-rate PE, fp32
storage): producers write f32r, DVE/ACT consumers read via bitcast to f32.
A ones-column appended to v (via host-built wv_aug/bv_aug) makes the attnV
matmul emit per-column E sums for free -> Z without activation accumulators.
"""

import sys

sys.path.insert(0, "/opt/trn_rl_repo")

import numpy as np
import ml_dtypes

import concourse.bass as bass
import concourse.mybir as mybir
import concourse.tile as tile
from concourse import bacc
from concourse.bass_utils import run_bass_kernel_spmd

F32 = mybir.dt.float32
F32R = mybir.dt.float32r
BF16 = mybir.dt.bfloat16
AF = mybir.ActivationFunctionType
OP = mybir.AluOpType
AX = mybir.AxisListType

B, S, D, H, DFF = 4, 1024, 512, 8, 2048
DEPTH = D // H  # 64
NEG = -1.0e9
EPS = 1e-9
N_CORES = 8
GROUPS = [[0, 1], [2, 3], [4, 5], [6, 7]]

SJ = S // 2       # own token half: 512
KD = D // 128     # 4 k-tiles over D
IT = S // 128     # 8 i tiles
KF = DFF // 128   # 16 dff tiles
VA = 2 * (4 * 65)  # v augmented with a ones column per head: 2 halves x 260


def _rd(ap):
    return ap.bitcast(F32)


def build():
    nc = bacc.Bacc("TRN2", target_bir_lowering=False, debug=False,
                   num_devices=N_CORES)

    def din(name, shape, dt=F32):
        return nc.dram_tensor(name, shape, dt, kind="ExternalInput").ap()

    xT = din("xT", [D, S], F32R)
    xb = din("xb", [D, S], BF16)
    maskp = din("maskp", [S, SJ], BF16)         # mask[:, own j] * (-1e9)
    ident = din("ident", [128, 128], BF16)
    wq = din("wq", [D, D], BF16)
    wk = din("wk", [D, D], BF16)
    wva = din("wva", [D, VA], BF16)             # v weights with ones-cols
    wo = din("wo", [D, D], BF16)
    w1 = din("w1", [D, DFF], BF16)
    w2 = din("w2", [DFF, D], BF16)
    bq = din("bq", [128, KD])
    bk = din("bk", [128, KD])
    bva = din("bva", [1, VA], BF16)
    bo = din("bo", [128, KD])
    b1 = din("b1", [128, KF])
    b2 = din("b2", [128, KD])
    g1 = din("g1", [128, KD])
    be1 = din("be1", [128, KD])
    g2 = din("g2", [128, KD])
    be2 = din("be2", [128, KD])
    id8 = din("id8", [8, 8])
    selp = din("selp", [8, 128], F32R)
    hout = nc.dram_tensor("hout", [D, SJ], F32, kind="ExternalOutput").ap()

    with tile.TileContext(nc) as tc:
        with (
            tc.tile_pool(name="const", bufs=1) as const,
            tc.tile_pool(name="state", bufs=1) as state,
            tc.tile_pool(name="scr", bufs=1) as scr,
            tc.tile_pool(name="psum", bufs=8, space="PSUM") as psum,
            tc.tile_pool(name="dram", bufs=1, space="DRAM") as dram,
        ):
            def loadw(name, src, kt, m, dt=BF16):
                t = const.tile([128, kt, m], dt, name=name, tag=name)
                nc.sync.dma_start(out=t, in_=src.rearrange("(k p) m -> p k m", p=128))
                return t

            def loadsm(name, src, dt=F32):
                t = const.tile(list(src.shape), dt, name=name, tag=name)
                nc.sync.dma_start(out=t, in_=src)
                return t

            # load order matters: the first qkv matmuls need x + wq/wk/wva
            h_cur = state.tile([128, KD, S], F32R, name="h0", tag="h", bufs=2)
            nc.sync.dma_start(out=h_cur, in_=xT.rearrange("(k p) s -> p k s", p=128))
            hb = state.tile([128, KD, S], BF16, name="hb0", tag="hb", bufs=2)
            nc.sync.dma_start(out=hb, in_=xb.rearrange("(k p) s -> p k s", p=128))
            wq_sb = loadw("wq_sb", wq, KD, D)
            wk_sb = loadw("wk_sb", wk, KD, D)
            wva_sb = loadw("wva_sb", wva, KD, VA)
            bq_sb = loadsm("bq_sb", bq)
            bk_sb = loadsm("bk_sb", bk)
            bva_sb = loadsm("bva_sb", bva, BF16)
            ones_sq = const.tile([128, 128], F32R, name="ones_sq", tag="ones_sq")
            nc.vector.memset(_rd(ones_sq), 1.0)
            ones_bf = const.tile([1, 128], BF16, name="ones_bf", tag="ones_bf")
            nc.vector.memset(ones_bf, 1.0)
            maskp_sb = const.tile([128, IT, SJ], BF16, name="maskp_sb", tag="maskp_sb")
            nc.sync.dma_start(out=maskp_sb, in_=maskp.rearrange("(i p) j -> p i j", p=128))
            id_sb = const.tile([128, 128], BF16, name="id_sb", tag="id_sb")
            nc.sync.dma_start(out=id_sb, in_=ident)
            wo_sb = loadw("wo_sb", wo, KD, D)
            w1_sb = loadw("w1_sb", w1, KD, DFF)
            bo_sb = loadsm("bo_sb", bo)
            b1_sb = loadsm("b1_sb", b1)
            b2_sb = loadsm("b2_sb", b2)
            g1_sb = loadsm("g1_sb", g1)
            be1_sb = loadsm("be1_sb", be1)
            g2_sb = loadsm("g2_sb", g2)
            be2_sb = loadsm("be2_sb", be2)
            eps_sb = const.tile([128, 1], F32, name="eps_sb", tag="eps_sb")
            nc.vector.memset(eps_sb, EPS)
            id8_sb = loadsm("id8_sb", id8)
            selp_sb = loadsm("selp_sb", selp, F32R)
            w2r = w2.rearrange("(k p) m -> p k m", p=128)

            def layernorm(z, g_sb, be_sb, out_name, out_tile):
                """z: f32r [128, KD, SJ]; writes normalized f32r into out_tile."""
                s1 = psum.tile([128, SJ], F32, name=f"s1_{out_name}", tag="ps")
                s2 = psum.tile([128, SJ], F32, name=f"s2_{out_name}", tag="ps")
                for k in range(KD):
                    sqc = scr.tile([128, SJ], F32R, name=f"sq_{out_name}_{k}",
                                   tag="e", bufs=4)
                    nc.scalar.activation(out=sqc, in_=_rd(z[:, k, :]), func=AF.Square)
                    nc.tensor.matmul(s1, lhsT=ones_sq, rhs=z[:, k, :],
                                     start=(k == 0), stop=(k == KD - 1))
                    nc.tensor.matmul(s2, lhsT=ones_sq, rhs=sqc,
                                     start=(k == 0), stop=(k == KD - 1))
                mean = scr.tile([128, SJ], F32, name=f"mean_{out_name}", tag="mean", bufs=1)
                rstd = scr.tile([128, SJ], F32, name=f"rstd_{out_name}", tag="rstd", bufs=1)
                nc.vector.tensor_scalar(out=mean, in0=s1, scalar1=1.0 / D,
                                        scalar2=None, op0=OP.mult)
                msq = scr.tile([128, SJ], F32, name=f"msq_{out_name}", tag="e", bufs=4)
                nc.vector.tensor_tensor(out=msq, in0=mean, in1=mean, op=OP.mult)
                var = scr.tile([128, SJ], F32, name=f"var_{out_name}", tag="e", bufs=4)
                nc.vector.scalar_tensor_tensor(out=var, in0=s2, scalar=1.0 / D,
                                               in1=msq, op0=OP.mult, op1=OP.subtract)
                nc.scalar.activation(out=var, in_=var, func=AF.Sqrt, bias=eps_sb[:, 0:1])
                nc.vector.reciprocal(out=rstd, in_=var)
                for k in range(KD):
                    t = scr.tile([128, SJ], F32, name=f"t_{out_name}_{k}",
                                 tag="e", bufs=4)
                    nc.vector.tensor_tensor(out=t, in0=_rd(z[:, k, :]), in1=mean,
                                            op=OP.subtract)
                    nc.vector.tensor_tensor(out=t, in0=t, in1=rstd, op=OP.mult)
                    nc.vector.tensor_scalar(out=out_tile[:, k, :], in0=t,
                                            scalar1=g_sb[:, k:k + 1],
                                            scalar2=be_sb[:, k:k + 1],
                                            op0=OP.mult, op1=OP.add)

            # --- phase helpers (closures over the current layer's tiles) ---
            def a_own(ly, qT, kT, v_sb, hb_t):
                """projections that need only the own (local-first) half of h"""
                for m in range(KD):
                    k_ps = psum.tile([128, SJ], F32, name=f"k_ps_{ly}_{m}", tag="ps")
                    for k in range(KD):
                        nc.tensor.matmul(k_ps, lhsT=wk_sb[:, k, m * 128:(m + 1) * 128],
                                         rhs=hb_t[:, k, 0:SJ],
                                         start=(k == 0), stop=(k == KD - 1))
                    nc.scalar.activation(out=kT[:, m, :], in_=k_ps,
                                         func=AF.Identity, bias=bk_sb[:, m:m + 1])
                for m in range(KD):
                    q_ps = psum.tile([128, SJ], F32, name=f"q_ps_{ly}_{m}_0", tag="ps")
                    for k in range(KD):
                        nc.tensor.matmul(q_ps, lhsT=wq_sb[:, k, m * 128:(m + 1) * 128],
                                         rhs=hb_t[:, k, 0:SJ],
                                         start=(k == 0), stop=(k == KD - 1))
                    nc.scalar.activation(out=qT[:, m, 0:SJ], in_=q_ps,
                                         func=AF.Identity, bias=bq_sb[:, m:m + 1])
                for it in range(IT // 2):
                    for hf in range(2):
                        v_ps = psum.tile([128, 260], F32, name=f"v_ps_{ly}_{it}_{hf}",
                                         tag="ps")
                        for k in range(KD):
                            nc.tensor.matmul(v_ps,
                                             lhsT=hb_t[:, k, it * 128:(it + 1) * 128],
                                             rhs=wva_sb[:, k, hf * 260:(hf + 1) * 260],
                                             start=(k == 0), stop=False)
                        nc.tensor.matmul(v_ps, lhsT=ones_bf,
                                         rhs=bva_sb[0:1, hf * 260:(hf + 1) * 260],
                                         start=False, stop=True)
                        nc.vector.tensor_copy(out=v_sb[:, it, hf, :], in_=v_ps)

            def a_peer(ly, qT, v_sb, hb_t):
                """projections needing the peer half of h"""
                for m in range(KD):
                    q_ps = psum.tile([128, SJ], F32, name=f"q_ps_{ly}_{m}_1", tag="ps")
                    for k in range(KD):
                        nc.tensor.matmul(q_ps, lhsT=wq_sb[:, k, m * 128:(m + 1) * 128],
                                         rhs=hb_t[:, k, SJ:S],
                                         start=(k == 0), stop=(k == KD - 1))
                    nc.scalar.activation(out=qT[:, m, SJ:S], in_=q_ps,
                                         func=AF.Identity, bias=bq_sb[:, m:m + 1])
                for it in range(IT // 2, IT):
                    for hf in range(2):
                        v_ps = psum.tile([128, 260], F32, name=f"v_ps_{ly}_{it}_{hf}",
                                         tag="ps")
                        for k in range(KD):
                            nc.tensor.matmul(v_ps,
                                             lhsT=hb_t[:, k, it * 128:(it + 1) * 128],
                                             rhs=wva_sb[:, k, hf * 260:(hf + 1) * 260],
                                             start=(k == 0), stop=False)
                        nc.tensor.matmul(v_ps, lhsT=ones_bf,
                                         rhs=bva_sb[0:1, hf * 260:(hf + 1) * 260],
                                         start=False, stop=True)
                        nc.vector.tensor_copy(out=v_sb[:, it, hf, :], in_=v_ps)

            def attn_pair(ly, hp, qT, kT, v_sb, outT, zparts):
                o_ps = [psum.tile([65, SJ], F32, name=f"o_ps_{ly}_{hp}_{hr}", tag="ps")
                        for hr in range(2)]
                for it in range(IT):
                    l_ps = [psum.tile([128, SJ], F32,
                                      name=f"l_ps_{ly}_{hp}_{it}_{hr}", tag="ps")
                            for hr in range(2)]
                    for hr in range(2):
                        nc.tensor.matmul(l_ps[hr], lhsT=id_sb,
                                         rhs=maskp_sb[:, it, :], start=True, stop=False)
                    for hr in range(2):
                        pb = 64 * hr
                        nc.tensor.matmul(l_ps[hr],
                                         lhsT=qT[pb:pb + 64, hp, it * 128:(it + 1) * 128],
                                         rhs=kT[pb:pb + 64, hp, :],
                                         start=False, stop=True)
                    for hr in range(2):
                        h_abs = 2 * hp + hr
                        e = scr.tile([128, SJ], BF16,
                                     name=f"e_{ly}_{hp}_{it}_{hr}", tag="e", bufs=4)
                        nc.scalar.activation(out=e, in_=l_ps[hr], func=AF.Exp,
                                             scale=0.125)
                        nc.tensor.matmul(
                            o_ps[hr],
                            lhsT=v_sb[:, it, h_abs // 4,
                                      65 * (h_abs % 4):65 * (h_abs % 4) + 65],
                            rhs=e, start=(it == 0), stop=(it == IT - 1))
                for hr in range(2):
                    h_abs = 2 * hp + hr
                    nc.vector.reduce_sum(out=zparts[64:65, h_abs:h_abs + 1],
                                         in_=o_ps[hr][64:65, :], axis=AX.X)
                    nc.scalar.activation(out=outT[64 * hr:64 * hr + 64, hp, :],
                                         in_=o_ps[hr][0:64, :], func=AF.Identity)

            def zcc_launch(ly, hp, zparts):
                ci = dram.tile([1, 2], F32, name=f"ccz_in_{ly}_{hp}",
                               tag=f"ccz_in_{ly}_{hp}")
                co = dram.tile([1, 2], F32, name=f"ccz_out_{ly}_{hp}",
                               tag=f"ccz_out_{ly}_{hp}")
                nc.sync.dma_start(out=ci, in_=zparts[64:65, 2 * hp:2 * hp + 2])
                nc.gpsimd.collective_compute("AllReduce", OP.add, replica_groups=GROUPS,
                                             ins=[ci.opt()], outs=[co.opt()])
                return co

            def zchain_c(ly, hp, co, outT, ap_ps):
                """1/Z for pair hp, scale outT pair, accumulate its out-proj part"""
                z2c = scr.tile([2, 1], F32, name=f"z2c_{ly}_{hp}", tag="z2c", bufs=2)
                nc.sync.dma_start(out=z2c, in_=bass.AP(tensor=co.tensor,
                                                       offset=co.offset,
                                                       ap=[[1, 2], [1, 1]]))
                z2i = scr.tile([2, 1], F32, name=f"z2i_{ly}_{hp}", tag="z2i", bufs=2)
                nc.vector.reciprocal(out=z2i, in_=z2c)
                dg2 = scr.tile([2, 2], F32R, name=f"dg2_{ly}_{hp}", tag="dg2", bufs=2)
                nc.vector.tensor_scalar(out=dg2, in0=id8_sb[0:2, 0:2], scalar1=z2i,
                                        scalar2=None, op0=OP.mult)
                zps = psum.tile([128, 2], F32, name=f"zps_{ly}_{hp}", tag="ps")
                nc.tensor.matmul(zps, lhsT=selp_sb[0:2, :], rhs=dg2,
                                 start=True, stop=True)
                zinv = scr.tile([128, 1], F32, name=f"zinv_{ly}_{hp}", tag="zinv", bufs=2)
                nc.vector.reduce_sum(out=zinv, in_=zps, axis=AX.X)
                nc.vector.tensor_scalar(out=outT[:, hp, :], in0=outT[:, hp, :],
                                        scalar1=zinv, scalar2=None, op0=OP.mult)
                for dt_ in range(KD):
                    nc.tensor.matmul(ap_ps[dt_],
                                     lhsT=wo_sb[:, hp, dt_ * 128:(dt_ + 1) * 128],
                                     rhs=outT[:, hp, :],
                                     start=(hp == 0), stop=(hp == KD - 1))

            # ================= main layer loop =================
            pend = None  # deferred boundary work (set at end of layer 0)
            for ly in range(2):
                qT = state.tile([128, KD, S], BF16, name=f"qT_{ly}", tag="qT", bufs=1)
                kT = state.tile([128, KD, SJ], BF16, name=f"kT_{ly}", tag="kT", bufs=1)
                v_sb = state.tile([128, IT, 2, 260], BF16, name=f"v_{ly}", tag="v",
                                  bufs=1)
                if ly == 0:
                    a_own(ly, qT, kT, v_sb, hb)
                    a_peer(ly, qT, v_sb, hb)
                else:
                    # own-half projections can start before the h-exchange lands
                    a_own(ly, qT, kT, v_sb, hb)
                    ccsum = state.tile([128, KD, SJ], F32, name="ccsum_0",
                                       tag="ccs", bufs=1)
                    nc.sync.dma_start(out=ccsum, in_=pend["ccs_out"].rearrange(
                        "(k p) s -> p k s", p=128))
                    h_next, hb_next = pend["h"], pend["hb"]
                    for k in range(KD):
                        nc.vector.tensor_tensor(out=h_next[:, k, SJ:S],
                                                in0=ccsum[:, k, :],
                                                in1=_rd(h_next[:, k, 0:SJ]),
                                                op=OP.subtract)
                    for k in range(KD):
                        nc.vector.tensor_copy(out=hb_next[:, k, SJ:S],
                                              in_=_rd(h_next[:, k, SJ:S]))
                    a_peer(ly, qT, v_sb, hb)

                # ---- B + Z pipeline + C accumulation ----
                outT = state.tile([128, KD, SJ], BF16, name=f"outT_{ly}", tag="outT",
                                  bufs=1)
                zparts = scr.tile([65, 8], F32, name=f"zp_{ly}", tag="zp", bufs=1)
                ap_ps = [psum.tile([128, SJ], F32, name=f"ap_ps_{ly}_{d}", tag="ps")
                         for d in range(KD)]
                cos = []
                for hp in range(KD):
                    attn_pair(ly, hp, qT, kT, v_sb, outT, zparts)
                    cos.append(zcc_launch(ly, hp, zparts))
                    if hp >= 1:
                        zchain_c(ly, hp - 1, cos[hp - 1], outT, ap_ps)
                zchain_c(ly, KD - 1, cos[KD - 1], outT, ap_ps)

                z1 = state.tile([128, KD, SJ], F32R, name=f"z1_{ly}", tag="qz", bufs=1)
                for dt_ in range(KD):
                    nc.vector.scalar_tensor_tensor(
                        out=z1[:, dt_, :], in0=ap_ps[dt_], scalar=bo_sb[:, dt_:dt_ + 1],
                        in1=_rd(h_cur[:, dt_, 0:SJ]), op0=OP.add, op1=OP.add)

                # ---- D: LN1 ----
                h1 = state.tile([128, KD, SJ], F32R, name=f"h1_{ly}", tag="h1", bufs=1)
                layernorm(z1, g1_sb, be1_sb, f"h1_{ly}", h1)
                h1b = state.tile([128, KD, SJ], BF16, name=f"h1b_{ly}", tag="h1b", bufs=1)
                for k in range(KD):
                    nc.vector.tensor_copy(out=h1b[:, k, :], in_=_rd(h1[:, k, :]))

                # ---- E: FFN (full DFF, own j-half), w2 streamed ----
                z2 = state.tile([128, KD, SJ], F32R, name=f"z2_{ly}", tag="qz", bufs=1)
                g_ps = [psum.tile([128, SJ], F32, name=f"g_ps_{ly}_{d}", tag="ps")
                        for d in range(KD)]
                for ft in range(KF):
                    w2c = scr.tile([128, D], BF16, name=f"w2c_{ly}_{ft}", tag="w2c",
                                   bufs=4)
                    nc.sync.dma_start(out=w2c, in_=w2r[:, ft, :])
                    f_ps = psum.tile([128, SJ], F32, name=f"f_ps_{ly}_{ft}", tag="ps")
                    for k in range(KD):
                        nc.tensor.matmul(f_ps, lhsT=w1_sb[:, k, ft * 128:(ft + 1) * 128],
                                         rhs=h1b[:, k, :],
                                         start=(k == 0), stop=(k == KD - 1))
                    fr = scr.tile([128, SJ], BF16, name=f"fr_{ly}_{ft}", tag="fr", bufs=3)
                    nc.scalar.activation(out=fr, in_=f_ps, func=AF.Relu,
                                         bias=b1_sb[:, ft:ft + 1])
                    for d in range(KD):
                        nc.tensor.matmul(g_ps[d], lhsT=w2c[:, d * 128:(d + 1) * 128],
                                         rhs=fr, start=(ft == 0), stop=(ft == KF - 1))
                for d in range(KD):
                    nc.vector.scalar_tensor_tensor(
                        out=z2[:, d, :], in0=g_ps[d], scalar=b2_sb[:, d:d + 1],
                        in1=_rd(h1[:, d, :]), op0=OP.add, op1=OP.add)

                # ---- F: LN2 -> exchange halves (or final output) ----
                if ly == 0:
                    h_next = state.tile([128, KD, S], F32R, name=f"h_{ly + 1}",
                                        tag="h", bufs=2)
                    layernorm(z2, g2_sb, be2_sb, f"hs_{ly}", h_next[:, :, 0:SJ])
                    ccs_in = dram.tile([D, SJ], F32, name=f"ccs_in_{ly}",
                                       tag=f"ccs_in_{ly}")
                    ccs_out = dram.tile([D, SJ], F32, name=f"ccs_out_{ly}",
                                        tag=f"ccs_out_{ly}")
                    nc.sync.dma_start(out=ccs_in.rearrange("(k p) s -> p k s", p=128),
                                      in_=_rd(h_next[:, :, 0:SJ]))
                    nc.gpsimd.collective_compute("AllReduce", OP.add,
                                                 replica_groups=GROUPS,
                                                 ins=[ccs_in.opt()], outs=[ccs_out.opt()])
                    hb_next = state.tile([128, KD, S], BF16, name=f"hb_{ly + 1}",
                                         tag="hb", bufs=2)
                    for k in range(KD):
                        nc.vector.tensor_copy(out=hb_next[:, k, 0:SJ],
                                              in_=_rd(h_next[:, k, 0:SJ]))
                    pend = {"ccs_out": ccs_out, "h": h_next, "hb": hb_next}
                    h_cur = h_next
                    hb = hb_next
                else:
                    hstage = state.tile([128, KD, SJ], F32R, name=f"hs_{ly}", tag="ccs",
                                        bufs=1)
                    layernorm(z2, g2_sb, be2_sb, f"hs_{ly}", hstage)
                    nc.sync.dma_start(out=hout.rearrange("(k p) s -> p k s", p=128),
                                      in_=_rd(hstage))

    nc.compile()
    return nc


_CACHE = {}


def _prep_inputs(x, mask, Wq, bq, Wk, bk, Wv, bv, Wo, bo, W1, b1, W2, b2,
                 g1, be1, g2, be2):
    f32 = np.float32
    x = np.asarray(x, f32)
    mask = np.asarray(mask, f32)
    ident = np.eye(128, dtype=ml_dtypes.bfloat16)

    Wv = np.asarray(Wv, f32)
    bv = np.asarray(bv, f32)
    wva = np.zeros((D, VA), f32)
    bva = np.zeros((1, VA), f32)
    for h in range(H):
        wva[:, 65 * h:65 * h + 64] = Wv[:, 64 * h:64 * h + 64]
        bva[0, 65 * h:65 * h + 64] = bv[64 * h:64 * h + 64]
        bva[0, 65 * h + 64] = 1.0

    def pp(v, cols):
        return np.ascontiguousarray(np.asarray(v, f32).reshape(cols, 128).T)

    selp = np.zeros((H, 128), f32)
    for h in range(H):
        selp[h, (h % 2) * 64:(h % 2) * 64 + 64] = 1.0

    bf16 = ml_dtypes.bfloat16
    common = {
        "id8": np.eye(H, dtype=f32),
        "selp": selp,
        "ident": ident,
        "wq": np.asarray(Wq, f32).astype(bf16),
        "wk": np.asarray(Wk, f32).astype(bf16),
        "wva": wva.astype(bf16),
        "wo": np.asarray(Wo, f32).astype(bf16),
        "w1": np.asarray(W1, f32).astype(bf16),
        "w2": np.asarray(W2, f32).astype(bf16),
        "bq": pp(bq, KD),
        "bk": pp(bk, KD),
        "bva": bva.astype(bf16),
        "bo": pp(bo, KD),
        "b1": pp(b1, KF),
        "b2": pp(b2, KD),
        "g1": pp(g1, KD),
        "be1": pp(be1, KD),
        "g2": pp(g2, KD),
        "be2": pp(be2, KD),
    }
    in_maps = []
    for c in range(N_CORES):
        b, r = c // 2, c % 2
        js = slice(r * SJ, (r + 1) * SJ)
        ps = slice((1 - r) * SJ, (2 - r) * SJ)
        # local token order: own half first (both in h columns and mask rows)
        xb = x[b].T
        m = dict(common)
        xtl = np.ascontiguousarray(np.concatenate([xb[:, js], xb[:, ps]], axis=1))
        m["xT"] = xtl
        m["xb"] = xtl.astype(bf16)
        mrows = np.concatenate([mask[b][js], mask[b][ps]], axis=0)
        m["maskp"] = np.ascontiguousarray(mrows[:, js] * NEG).astype(ml_dtypes.bfloat16)
        in_maps.append(m)
    return in_maps


def get_nc():
    if "nc" not in _CACHE:
        _CACHE["nc"] = build()
    return _CACHE["nc"]


def run(in_maps, **kw):
    nc = get_nc()
    return run_bass_kernel_spmd(nc, in_maps, core_ids=list(range(N_CORES)), **kw)


def kernel(**inputs):
    in_maps = _prep_inputs(**inputs)
    res = run(in_maps)
    out = np.empty((B, S, D), np.float32)
    for c in range(N_CORES):
        b, r = c // 2, c % 2
        out[b, r * SJ:(r + 1) * SJ, :] = res.results[c]["hout"].T
    return out


# revision 33
# speedup vs baseline: 1.4345x; 1.4345x over previous
"""Trainium2 Bass kernel for nn_Encoders (2-layer shared-weight transformer encoder).

Sharding (v3): 8 cores; pair (2b, 2b+1) handles batch b.  Within a pair the
split is along the attention *output* token axis j (the reference's unusual
attention contracts over queries i: out[j,d] = sum_i attn[i,j] v[i,d]):

  - each core computes q and v for ALL tokens/heads (small duplication),
    k only for its own j-half,
  - E = exp((qk^T + mask*NEG)/8) for its own j columns, all heads,
  - attention output, out-projection, residual+LN1, full-DFF FFN,
    residual+LN2 for its own j-half only -- NO partial-sum collectives.

Cross-core data: the joint-softmax denominator Z (per head) is summed with a
32-byte AllReduce, and the layer output h is AllGathered (1MB) at the layer
boundary; the final layer outputs each core's own half directly.

Everything stays in transposed layout [feature, token]; LN stats via all-ones
matmul (broadcast sums).  Matmul inputs are float32r (full-rate PE, fp32
storage): producers write f32r, DVE/ACT consumers read via bitcast to f32.
A ones-column appended to v (via host-built wv_aug/bv_aug) makes the attnV
matmul emit per-column E sums for free -> Z without activation accumulators.
"""

import sys

sys.path.insert(0, "/opt/trn_rl_repo")

import numpy as np
import ml_dtypes

import concourse.bass as bass
import concourse.mybir as mybir
import concourse.tile as tile
from concourse import bacc
from concourse.bass_utils import run_bass_kernel_spmd

F32 = mybir.dt.float32
F32R = mybir.dt.float32r
BF16 = mybir.dt.bfloat16
AF = mybir.ActivationFunctionType
OP = mybir.AluOpType
AX = mybir.AxisListType

B, S, D, H, DFF = 4, 1024, 512, 8, 2048
DEPTH = D // H  # 64
NEG = -1.0e9
EPS = 1e-9
N_CORES = 8
GROUPS = [[0, 1], [2, 3], [4, 5], [6, 7]]

SJ = S // 2       # own token half: 512
KD = D // 128     # 4 k-tiles over D
IT = S // 128     # 8 i tiles
KF = DFF // 128   # 16 dff tiles
VA = 2 * (4 * 65)  # v augmented with a ones column per head: 2 halves x 260


def _rd(ap):
    return ap.bitcast(F32)


def build():
    nc = bacc.Bacc("TRN2", target_bir_lowering=False, debug=False,
                   num_devices=N_CORES)

    def din(name, shape, dt=F32):
        return nc.dram_tensor(name, shape, dt, kind="ExternalInput").ap()

    xT = din("xT", [D, S], F32R)
    xb = din("xb", [D, S], BF16)
    maskp = din("maskp", [S, SJ], BF16)         # mask[:, own j] * (-1e9)
    ident = din("ident", [128, 128], BF16)
    wq = din("wq", [D, D], BF16)
    wk = din("wk", [D, D], BF16)
    wva = din("wva", [D, VA], BF16)             # v weights with ones-cols
    wo = din("wo", [D, D], BF16)
    w1 = din("w1", [D, DFF], BF16)
    w2 = din("w2", [DFF, D], BF16)
    bq = din("bq", [128, KD])
    bk = din("bk", [128, KD])
    bva = din("bva", [1, VA], BF16)
    bo = din("bo", [128, KD])
    b1 = din("b1", [128, KF])
    b2 = din("b2", [128, KD])
    g1 = din("g1", [128, KD])
    be1 = din("be1", [128, KD])
    g2 = din("g2", [128, KD])
    be2 = din("be2", [128, KD])
    id8 = din("id8", [8, 8])
    selp = din("selp", [8, 128], F32R)
    hout = nc.dram_tensor("hout", [D, SJ], F32, kind="ExternalOutput").ap()

    with tile.TileContext(nc) as tc:
        with (
            tc.tile_pool(name="const", bufs=1) as const,
            tc.tile_pool(name="state", bufs=1) as state,
            tc.tile_pool(name="scr", bufs=1) as scr,
            tc.tile_pool(name="psum", bufs=8, space="PSUM") as psum,
            tc.tile_pool(name="dram", bufs=1, space="DRAM") as dram,
        ):
            def loadw(name, src, kt, m, dt=BF16):
                t = const.tile([128, kt, m], dt, name=name, tag=name)
                nc.sync.dma_start(out=t, in_=src.rearrange("(k p) m -> p k m", p=128))
                return t

            def loadsm(name, src, dt=F32):
                t = const.tile(list(src.shape), dt, name=name, tag=name)
                nc.sync.dma_start(out=t, in_=src)
                return t

            # load order matters: the first qkv matmuls need x + wq/wk/wva
            h_cur = state.tile([128, KD, S], F32R, name="h0", tag="h", bufs=2)
            nc.sync.dma_start(out=h_cur, in_=xT.rearrange("(k p) s -> p k s", p=128))
            hb = state.tile([128, KD, S], BF16, name="hb0", tag="hb", bufs=2)
            nc.sync.dma_start(out=hb, in_=xb.rearrange("(k p) s -> p k s", p=128))
            wq_sb = loadw("wq_sb", wq, KD, D)
            wk_sb = loadw("wk_sb", wk, KD, D)
            wva_sb = loadw("wva_sb", wva, KD, VA)
            bq_sb = loadsm("bq_sb", bq)
            bk_sb = loadsm("bk_sb", bk)
            bva_sb = loadsm("bva_sb", bva, BF16)
            ones_sq = const.tile([128, 128], F32R, name="ones_sq", tag="ones_sq")
            nc.vector.memset(_rd(ones_sq), 1.0)
            ones_bf = const.tile([1, 128], BF16, name="ones_bf", tag="ones_bf")
            nc.vector.memset(ones_bf, 1.0)
            maskp_sb = const.tile([128, IT, SJ], BF16, name="maskp_sb", tag="maskp_sb")
            nc.sync.dma_start(out=maskp_sb, in_=maskp.rearrange("(i p) j -> p i j", p=128))
            id_sb = const.tile([128, 128], BF16, name="id_sb", tag="id_sb")
            nc.sync.dma_start(out=id_sb, in_=ident)
            wo_sb = loadw("wo_sb", wo, KD, D)
            w1_sb = loadw("w1_sb", w1, KD, DFF)
            bo_sb = loadsm("bo_sb", bo)
            b1_sb = loadsm("b1_sb", b1)
            b2_sb = loadsm("b2_sb", b2)
            g1_sb = loadsm("g1_sb", g1)
            be1_sb = loadsm("be1_sb", be1)
            g2_sb = loadsm("g2_sb", g2)
            be2_sb = loadsm("be2_sb", be2)
            eps_sb = const.tile([128, 1], F32, name="eps_sb", tag="eps_sb")
            nc.vector.memset(eps_sb, EPS)
            id8_sb = loadsm("id8_sb", id8)
            selp_sb = loadsm("selp_sb", selp, F32R)
            w2r = w2.rearrange("(k p) m -> p k m", p=128)

            def layernorm(z, g_sb, be_sb, out_name, out_tile):
                """z: f32r [128, KD, SJ]; writes normalized f32r into out_tile."""
                s1 = psum.tile([128, SJ], F32, name=f"s1_{out_name}", tag="ps")
                s2 = psum.tile([128, SJ], F32, name=f"s2_{out_name}", tag="ps")
                for k in range(KD):
                    sqc = scr.tile([128, SJ], F32R, name=f"sq_{out_name}_{k}",
                                   tag="e", bufs=4)
                    nc.scalar.activation(out=sqc, in_=_rd(z[:, k, :]), func=AF.Square)
                    nc.tensor.matmul(s1, lhsT=ones_sq, rhs=z[:, k, :],
                                     start=(k == 0), stop=(k == KD - 1))
                    nc.tensor.matmul(s2, lhsT=ones_sq, rhs=sqc,
                                     start=(k == 0), stop=(k == KD - 1))
                mean = scr.tile([128, SJ], F32, name=f"mean_{out_name}", tag="mean", bufs=1)
                rstd = scr.tile([128, SJ], F32, name=f"rstd_{out_name}", tag="rstd", bufs=1)
                nc.vector.tensor_scalar(out=mean, in0=s1, scalar1=1.0 / D,
                                        scalar2=None, op0=OP.mult)
                msq = scr.tile([128, SJ], F32, name=f"msq_{out_name}", tag="e", bufs=4)
                nc.vector.tensor_tensor(out=msq, in0=mean, in1=mean, op=OP.mult)
                var = scr.tile([128, SJ], F32, name=f"var_{out_name}", tag="e", bufs=4)
                nc.vector.scalar_tensor_tensor(out=var, in0=s2, scalar=1.0 / D,
                                               in1=msq, op0=OP.mult, op1=OP.subtract)
                nc.scalar.activation(out=var, in_=var, func=AF.Sqrt, bias=eps_sb[:, 0:1])
                nc.vector.reciprocal(out=rstd, in_=var)
                for k in range(KD):
                    t = scr.tile([128, SJ], F32, name=f"t_{out_name}_{k}",
                                 tag="e", bufs=4)
                    nc.vector.tensor_tensor(out=t, in0=_rd(z[:, k, :]), in1=mean,
                                            op=OP.subtract)
                    nc.vector.tensor_tensor(out=t, in0=t, in1=rstd, op=OP.mult)
                    nc.vector.tensor_scalar(out=out_tile[:, k, :], in0=t,
                                            scalar1=g_sb[:, k:k + 1],
                                            scalar2=be_sb[:, k:k + 1],
                                            op0=OP.mult, op1=OP.add)

            # --- phase helpers (closures over the current layer's tiles) ---
            def a_own(ly, qT, kT, v_sb, hb_t):
                """projections that need only the own (local-first) half of h"""
                for m in range(KD):
                    k_ps = psum.tile([128, SJ], F32, name=f"k_ps_{ly}_{m}", tag="ps")
                    for k in range(KD):
                        nc.tensor.matmul(k_ps, lhsT=wk_sb[:, k, m * 128:(m + 1) * 128],
                                         rhs=hb_t[:, k, 0:SJ],
                                         start=(k == 0), stop=(k == KD - 1))
                    nc.scalar.activation(out=kT[:, m, :], in_=k_ps,
                                         func=AF.Identity, bias=bk_sb[:, m:m + 1])
                for m in range(KD):
                    q_ps = psum.tile([128, SJ], F32, name=f"q_ps_{ly}_{m}_0", tag="ps")
                    for k in range(KD):
                        nc.tensor.matmul(q_ps, lhsT=wq_sb[:, k, m * 128:(m + 1) * 128],
                                         rhs=hb_t[:, k, 0:SJ],
                                         start=(k == 0), stop=(k == KD - 1))
                    nc.scalar.activation(out=qT[:, m, 0:SJ], in_=q_ps,
                                         func=AF.Identity, bias=bq_sb[:, m:m + 1])
                for it in range(IT // 2):
                    for hf in range(2):
                        v_ps = psum.tile([128, 260], F32, name=f"v_ps_{ly}_{it}_{hf}",
                                         tag="ps")
                        for k in range(KD):
                            nc.tensor.matmul(v_ps,
                                             lhsT=hb_t[:, k, it * 128:(it + 1) * 128],
                                             rhs=wva_sb[:, k, hf * 260:(hf + 1) * 260],
                                             start=(k == 0), stop=False)
                        nc.tensor.matmul(v_ps, lhsT=ones_bf,
                                         rhs=bva_sb[0:1, hf * 260:(hf + 1) * 260],
                                         start=False, stop=True)
                        nc.vector.tensor_copy(out=v_sb[:, it, hf, :], in_=v_ps)

            def a_peer(ly, qT, v_sb, hb_t):
                """projections needing the peer half of h"""
                for m in range(KD):
                    q_ps = psum.tile([128, SJ], F32, name=f"q_ps_{ly}_{m}_1", tag="ps")
                    for k in range(KD):
                        nc.tensor.matmul(q_ps, lhsT=wq_sb[:, k, m * 128:(m + 1) * 128],
                                         rhs=hb_t[:, k, SJ:S],
                                         start=(k == 0), stop=(k == KD - 1))
                    nc.scalar.activation(out=qT[:, m, SJ:S], in_=q_ps,
                                         func=AF.Identity, bias=bq_sb[:, m:m + 1])
                for it in range(IT // 2, IT):
                    for hf in range(2):
                        v_ps = psum.tile([128, 260], F32, name=f"v_ps_{ly}_{it}_{hf}",
                                         tag="ps")
                        for k in range(KD):
                            nc.tensor.matmul(v_ps,
                                             lhsT=hb_t[:, k, it * 128:(it + 1) * 128],
                                             rhs=wva_sb[:, k, hf * 260:(hf + 1) * 260],
                                             start=(k == 0), stop=False)
                        nc.tensor.matmul(v_ps, lhsT=ones_bf,
                                         rhs=bva_sb[0:1, hf * 260:(hf + 1) * 260],
                                         start=False, stop=True)
                        nc.vector.tensor_copy(out=v_sb[:, it, hf, :], in_=v_ps)

            def attn_pair(ly, hp, qT, kT, v_sb, outT, zparts):
                o_ps = [psum.tile([65, SJ], F32, name=f"o_ps_{ly}_{hp}_{hr}", tag="ps")
                        for hr in range(2)]
                for it in range(IT):
                    l_ps = [psum.tile([128, SJ], F32,
                                      name=f"l_ps_{ly}_{hp}_{it}_{hr}", tag="ps")
                            for hr in range(2)]
                    for hr in range(2):
                        nc.tensor.matmul(l_ps[hr], lhsT=id_sb,
                                         rhs=maskp_sb[:, it, :], start=True, stop=False)
                    for hr in range(2):
                        pb = 64 * hr
                        nc.tensor.matmul(l_ps[hr],
                                         lhsT=qT[pb:pb + 64, hp, it * 128:(it + 1) * 128],
                                         rhs=kT[pb:pb + 64, hp, :],
                                         start=False, stop=True)
                    for hr in range(2):
                        h_abs = 2 * hp + hr
                        e = scr.tile([128, SJ], BF16,
                                     name=f"e_{ly}_{hp}_{it}_{hr}", tag="e", bufs=4)
                        nc.scalar.activation(out=e, in_=l_ps[hr], func=AF.Exp,
                                             scale=0.125)
                        nc.tensor.matmul(
                            o_ps[hr],
                            lhsT=v_sb[:, it, h_abs // 4,
                                      65 * (h_abs % 4):65 * (h_abs % 4) + 65],
                            rhs=e, start=(it == 0), stop=(it == IT - 1))
                for hr in range(2):
                    h_abs = 2 * hp + hr
                    nc.vector.reduce_sum(out=zparts[64:65, h_abs:h_abs + 1],
                                         in_=o_ps[hr][64:65, :], axis=AX.X)
                    nc.scalar.activation(out=outT[64 * hr:64 * hr + 64, hp, :],
                                         in_=o_ps[hr][0:64, :], func=AF.Identity)

            def zcc_launch(ly, hp, zparts):
                ci = dram.tile([1, 2], F32, name=f"ccz_in_{ly}_{hp}",
                               tag=f"ccz_in_{ly}_{hp}")
                co = dram.tile([1, 2], F32, name=f"ccz_out_{ly}_{hp}",
                               tag=f"ccz_out_{ly}_{hp}")
                nc.sync.dma_start(out=ci, in_=zparts[64:65, 2 * hp:2 * hp + 2])
                nc.gpsimd.collective_compute("AllReduce", OP.add, replica_groups=GROUPS,
                                             ins=[ci.opt()], outs=[co.opt()])
                return co

            def zchain_c(ly, hp, co, outT, ap_ps):
                """1/Z for pair hp, scale outT pair, accumulate its out-proj part"""
                z2c = scr.tile([2, 1], F32, name=f"z2c_{ly}_{hp}", tag="z2c", bufs=2)
                nc.sync.dma_start(out=z2c, in_=bass.AP(tensor=co.tensor,
                                                       offset=co.offset,
                                                       ap=[[1, 2], [1, 1]]))
                z2i = scr.tile([2, 1], F32, name=f"z2i_{ly}_{hp}", tag="z2i", bufs=2)
                nc.vector.reciprocal(out=z2i, in_=z2c)
                dg2 = scr.tile([2, 2], F32R, name=f"dg2_{ly}_{hp}", tag="dg2", bufs=2)
                nc.vector.tensor_scalar(out=dg2, in0=id8_sb[0:2, 0:2], scalar1=z2i,
                                        scalar2=None, op0=OP.mult)
                zps = psum.tile([128, 2], F32, name=f"zps_{ly}_{hp}", tag="ps")
                nc.tensor.matmul(zps, lhsT=selp_sb[0:2, :], rhs=dg2,
                                 start=True, stop=True)
                zinv = scr.tile([128, 1], F32, name=f"zinv_{ly}_{hp}", tag="zinv", bufs=2)
                nc.vector.reduce_sum(out=zinv, in_=zps, axis=AX.X)
                nc.vector.tensor_scalar(out=outT[:, hp, :], in0=outT[:, hp, :],
                                        scalar1=zinv, scalar2=None, op0=OP.mult)
                for dt_ in range(KD):
                    nc.tensor.matmul(ap_ps[dt_],
                                     lhsT=wo_sb[:, hp, dt_ * 128:(dt_ + 1) * 128],
                                     rhs=outT[:, hp, :],
                                     start=(hp == 0), stop=(hp == KD - 1))

            # ================= main layer loop =================
            pend = None  # deferred boundary work (set at end of layer 0)
            for ly in range(2):
                qT = state.tile([128, KD, S], BF16, name=f"qT_{ly}", tag="qT", bufs=1)
                kT = state.tile([128, KD, SJ], BF16, name=f"kT_{ly}", tag="kT", bufs=1)
                v_sb = state.tile([128, IT, 2, 260], BF16, name=f"v_{ly}", tag="v",
                                  bufs=1)
                if ly == 0:
                    a_own(ly, qT, kT, v_sb, hb)
                    a_peer(ly, qT, v_sb, hb)
                else:
                    # own-half projections can start before the h-exchange lands
                    a_own(ly, qT, kT, v_sb, hb)
                    ccsum = state.tile([128, KD, SJ], F32, name="ccsum_0",
                                       tag="ccs", bufs=1)
                    nc.sync.dma_start(out=ccsum, in_=pend["ccs_out"].rearrange(
                        "(k p) s -> p k s", p=128))
                    h_next, hb_next = pend["h"], pend["hb"]
                    for k in range(KD):
                        nc.vector.tensor_tensor(out=h_next[:, k, SJ:S],
                                                in0=ccsum[:, k, :],
                                                in1=_rd(h_next[:, k, 0:SJ]),
                                                op=OP.subtract)
                    for k in range(KD):
                        nc.vector.tensor_copy(out=hb_next[:, k, SJ:S],
                                              in_=_rd(h_next[:, k, SJ:S]))
                    a_peer(ly, qT, v_sb, hb)

                # ---- B + Z pipeline + C accumulation ----
                outT = state.tile([128, KD, SJ], BF16, name=f"outT_{ly}", tag="outT",
                                  bufs=1)
                zparts = scr.tile([65, 8], F32, name=f"zp_{ly}", tag="zp", bufs=1)
                ap_ps = [psum.tile([128, SJ], F32, name=f"ap_ps_{ly}_{d}", tag="ps")
                         for d in range(KD)]
                cos = []
                for hp in range(KD):
                    attn_pair(ly, hp, qT, kT, v_sb, outT, zparts)
                    cos.append(zcc_launch(ly, hp, zparts))
                for hp in range(KD):
                    zchain_c(ly, hp, cos[hp], outT, ap_ps)

                z1 = state.tile([128, KD, SJ], F32R, name=f"z1_{ly}", tag="qz", bufs=1)
                for dt_ in range(KD):
                    nc.vector.scalar_tensor_tensor(
                        out=z1[:, dt_, :], in0=ap_ps[dt_], scalar=bo_sb[:, dt_:dt_ + 1],
                        in1=_rd(h_cur[:, dt_, 0:SJ]), op0=OP.add, op1=OP.add)

                # ---- D: LN1 ----
                h1 = state.tile([128, KD, SJ], F32R, name=f"h1_{ly}", tag="h1", bufs=1)
                layernorm(z1, g1_sb, be1_sb, f"h1_{ly}", h1)
                h1b = state.tile([128, KD, SJ], BF16, name=f"h1b_{ly}", tag="h1b", bufs=1)
                for k in range(KD):
                    nc.vector.tensor_copy(out=h1b[:, k, :], in_=_rd(h1[:, k, :]))

                # ---- E: FFN (full DFF, own j-half), w2 streamed ----
                z2 = state.tile([128, KD, SJ], F32R, name=f"z2_{ly}", tag="qz", bufs=1)
                g_ps = [psum.tile([128, SJ], F32, name=f"g_ps_{ly}_{d}", tag="ps")
                        for d in range(KD)]
                for ft in range(KF):
                    w2c = scr.tile([128, D], BF16, name=f"w2c_{ly}_{ft}", tag="w2c",
                                   bufs=4)
                    nc.sync.dma_start(out=w2c, in_=w2r[:, ft, :])
                    f_ps = psum.tile([128, SJ], F32, name=f"f_ps_{ly}_{ft}", tag="ps")
                    for k in range(KD):
                        nc.tensor.matmul(f_ps, lhsT=w1_sb[:, k, ft * 128:(ft + 1) * 128],
                                         rhs=h1b[:, k, :],
                                         start=(k == 0), stop=(k == KD - 1))
                    fr = scr.tile([128, SJ], BF16, name=f"fr_{ly}_{ft}", tag="fr", bufs=3)
                    nc.scalar.activation(out=fr, in_=f_ps, func=AF.Relu,
                                         bias=b1_sb[:, ft:ft + 1])
                    for d in range(KD):
                        nc.tensor.matmul(g_ps[d], lhsT=w2c[:, d * 128:(d + 1) * 128],
                                         rhs=fr, start=(ft == 0), stop=(ft == KF - 1))
                for d in range(KD):
                    nc.vector.scalar_tensor_tensor(
                        out=z2[:, d, :], in0=g_ps[d], scalar=b2_sb[:, d:d + 1],
                        in1=_rd(h1[:, d, :]), op0=OP.add, op1=OP.add)

                # ---- F: LN2 -> exchange halves (or final output) ----
                if ly == 0:
                    h_next = state.tile([128, KD, S], F32R, name=f"h_{ly + 1}",
                                        tag="h", bufs=2)
                    layernorm(z2, g2_sb, be2_sb, f"hs_{ly}", h_next[:, :, 0:SJ])
                    ccs_in = dram.tile([D, SJ], F32, name=f"ccs_in_{ly}",
                                       tag=f"ccs_in_{ly}")
                    ccs_out = dram.tile([D, SJ], F32, name=f"ccs_out_{ly}",
                                        tag=f"ccs_out_{ly}")
                    nc.sync.dma_start(out=ccs_in.rearrange("(k p) s -> p k s", p=128),
                                      in_=_rd(h_next[:, :, 0:SJ]))
                    nc.gpsimd.collective_compute("AllReduce", OP.add,
                                                 replica_groups=GROUPS,
                                                 ins=[ccs_in.opt()], outs=[ccs_out.opt()])
                    hb_next = state.tile([128, KD, S], BF16, name=f"hb_{ly + 1}",
                                         tag="hb", bufs=2)
                    for k in range(KD):
                        nc.vector.tensor_copy(out=hb_next[:, k, 0:SJ],
                                              in_=_rd(h_next[:, k, 0:SJ]))
                    pend = {"ccs_out": ccs_out, "h": h_next, "hb": hb_next}
                    h_cur = h_next
                    hb = hb_next
                else:
                    hstage = state.tile([128, KD, SJ], F32R, name=f"hs_{ly}", tag="ccs",
                                        bufs=1)
                    layernorm(z2, g2_sb, be2_sb, f"hs_{ly}", hstage)
                    nc.sync.dma_start(out=hout.rearrange("(k p) s -> p k s", p=128),
                                      in_=_rd(hstage))

    nc.compile()
    return nc


_CACHE = {}


def _prep_inputs(x, mask, Wq, bq, Wk, bk, Wv, bv, Wo, bo, W1, b1, W2, b2,
                 g1, be1, g2, be2):
    f32 = np.float32
    x = np.asarray(x, f32)
    mask = np.asarray(mask, f32)
    ident = np.eye(128, dtype=ml_dtypes.bfloat16)

    Wv = np.asarray(Wv, f32)
    bv = np.asarray(bv, f32)
    wva = np.zeros((D, VA), f32)
    bva = np.zeros((1, VA), f32)
    for h in range(H):
        wva[:, 65 * h:65 * h + 64] = Wv[:, 64 * h:64 * h + 64]
        bva[0, 65 * h:65 * h + 64] = bv[64 * h:64 * h + 64]
        bva[0, 65 * h + 64] = 1.0

    def pp(v, cols):
        return np.ascontiguousarray(np.asarray(v, f32).reshape(cols, 128).T)

    selp = np.zeros((H, 128), f32)
    for h in range(H):
        selp[h, (h % 2) * 64:(h % 2) * 64 + 64] = 1.0

    bf16 = ml_dtypes.bfloat16
    common = {
        "id8": np.eye(H, dtype=f32),
        "selp": selp,
        "ident": ident,
        "wq": np.asarray(Wq, f32).astype(bf16),
        "wk": np.asarray(Wk, f32).astype(bf16),
        "wva": wva.astype(bf16),
        "wo": np.asarray(Wo, f32).astype(bf16),
        "w1": np.asarray(W1, f32).astype(bf16),
        "w2": np.asarray(W2, f32).astype(bf16),
        "bq": pp(bq, KD),
        "bk": pp(bk, KD),
        "bva": bva.astype(bf16),
        "bo": pp(bo, KD),
        "b1": pp(b1, KF),
        "b2": pp(b2, KD),
        "g1": pp(g1, KD),
        "be1": pp(be1, KD),
        "g2": pp(g2, KD),
        "be2": pp(be2, KD),
    }
    in_maps = []
    for c in range(N_CORES):
        b, r = c // 2, c % 2
        js = slice(r * SJ, (r + 1) * SJ)
        ps = slice((1 - r) * SJ, (2 - r) * SJ)
        # local token order: own half first (both in h columns and mask rows)
        xb = x[b].T
        m = dict(common)
        xtl = np.ascontiguousarray(np.concatenate([xb[:, js], xb[:, ps]], axis=1))
        m["xT"] = xtl
        m["xb"] = xtl.astype(bf16)
        mrows = np.concatenate([mask[b][js], mask[b][ps]], axis=0)
        m["maskp"] = np.ascontiguousarray(mrows[:, js] * NEG).astype(ml_dtypes.bfloat16)
        in_maps.append(m)
    return in_maps


def get_nc():
    if "nc" not in _CACHE:
        _CACHE["nc"] = build()
    return _CACHE["nc"]


def run(in_maps, **kw):
    nc = get_nc()
    return run_bass_kernel_spmd(nc, in_maps, core_ids=list(range(N_CORES)), **kw)


def kernel(**inputs):
    in_maps = _prep_inputs(**inputs)
    res = run(in_maps)
    out = np.empty((B, S, D), np.float32)
    for c in range(N_CORES):
        b, r = c // 2, c % 2
        out[b, r * SJ:(r + 1) * SJ, :] = res.results[c]["hout"].T
    return out


# revision 35
# speedup vs baseline: 1.5996x; 1.1151x over previous
"""Trainium2 Bass kernel for nn_Encoders (2-layer shared-weight transformer encoder).

Sharding (v3): 8 cores; pair (2b, 2b+1) handles batch b.  Within a pair the
split is along the attention *output* token axis j (the reference's unusual
attention contracts over queries i: out[j,d] = sum_i attn[i,j] v[i,d]):

  - each core computes q and v for ALL tokens/heads (small duplication),
    k only for its own j-half,
  - E = exp((qk^T + mask*NEG)/8) for its own j columns, all heads,
  - attention output, out-projection, residual+LN1, full-DFF FFN,
    residual+LN2 for its own j-half only -- NO partial-sum collectives.

Cross-core data: the joint-softmax denominator Z (per head) is summed with a
32-byte AllReduce, and the layer output h is AllGathered (1MB) at the layer
boundary; the final layer outputs each core's own half directly.

Everything stays in transposed layout [feature, token]; LN stats via all-ones
matmul (broadcast sums).  Matmul inputs are float32r (full-rate PE, fp32
storage): producers write f32r, DVE/ACT consumers read via bitcast to f32.
A ones-column appended to v (via host-built wv_aug/bv_aug) makes the attnV
matmul emit per-column E sums for free -> Z without activation accumulators.
"""

import sys

sys.path.insert(0, "/opt/trn_rl_repo")

import numpy as np
import ml_dtypes

import concourse.bass as bass
import concourse.mybir as mybir
import concourse.tile as tile
from concourse import bacc
from concourse.bass_utils import run_bass_kernel_spmd

F32 = mybir.dt.float32
F32R = mybir.dt.float32r
BF16 = mybir.dt.bfloat16
AF = mybir.ActivationFunctionType
OP = mybir.AluOpType
AX = mybir.AxisListType

B, S, D, H, DFF = 4, 1024, 512, 8, 2048
DEPTH = D // H  # 64
NEG = -1.0e9
EPS = 1e-9
N_CORES = 8
GROUPS = [[0, 1], [2, 3], [4, 5], [6, 7]]

SJ = S // 2       # own token half: 512
KD = D // 128     # 4 k-tiles over D
IT = S // 128     # 8 i tiles
KF = DFF // 128   # 16 dff tiles
VA = 2 * (4 * 65)  # v augmented with a ones column per head: 2 halves x 260


def _rd(ap):
    return ap.bitcast(F32)


def build():
    nc = bacc.Bacc("TRN2", target_bir_lowering=False, debug=False,
                   num_devices=N_CORES)

    def din(name, shape, dt=F32):
        return nc.dram_tensor(name, shape, dt, kind="ExternalInput").ap()

    xT = din("xT", [D, S], F32R)
    xb = din("xb", [D, S], BF16)
    maskp = din("maskp", [S, SJ], BF16)         # mask[:, own j] * (-1e9)
    ident = din("ident", [128, 128], BF16)
    wq = din("wq", [D, D], BF16)
    wk = din("wk", [D, D], BF16)
    wva = din("wva", [D, VA], BF16)             # v weights with ones-cols
    wo = din("wo", [D, D], BF16)
    w1 = din("w1", [D, DFF], BF16)
    w2 = din("w2", [DFF, D], BF16)
    bq = din("bq", [128, KD])
    bk = din("bk", [128, KD])
    bva = din("bva", [1, VA], BF16)
    bo = din("bo", [128, KD])
    b1 = din("b1", [128, KF])
    b2 = din("b2", [128, KD])
    g1 = din("g1", [128, KD])
    be1 = din("be1", [128, KD])
    g2 = din("g2", [128, KD])
    be2 = din("be2", [128, KD])
    id8 = din("id8", [8, 8])
    selp = din("selp", [8, 128], F32R)
    hout = nc.dram_tensor("hout", [D, SJ], F32, kind="ExternalOutput").ap()

    with tile.TileContext(nc) as tc:
        with (
            tc.tile_pool(name="const", bufs=1) as const,
            tc.tile_pool(name="state", bufs=1) as state,
            tc.tile_pool(name="scr", bufs=1) as scr,
            tc.tile_pool(name="psum", bufs=4, space="PSUM") as psum,
            tc.tile_pool(name="dram", bufs=1, space="DRAM") as dram,
        ):
            def loadw(name, src, kt, m, dt=BF16):
                t = const.tile([128, kt, m], dt, name=name, tag=name)
                nc.sync.dma_start(out=t, in_=src.rearrange("(k p) m -> p k m", p=128))
                return t

            def loadsm(name, src, dt=F32):
                t = const.tile(list(src.shape), dt, name=name, tag=name)
                nc.sync.dma_start(out=t, in_=src)
                return t

            # load order matters: the first qkv matmuls need x + wq/wk/wva
            h_cur = state.tile([128, KD, S], F32R, name="h0", tag="h", bufs=2)
            nc.sync.dma_start(out=h_cur, in_=xT.rearrange("(k p) s -> p k s", p=128))
            hb = state.tile([128, KD, S], BF16, name="hb0", tag="hb", bufs=2)
            nc.sync.dma_start(out=hb, in_=xb.rearrange("(k p) s -> p k s", p=128))
            wq_sb = loadw("wq_sb", wq, KD, D)
            wk_sb = loadw("wk_sb", wk, KD, D)
            wva_sb = loadw("wva_sb", wva, KD, VA)
            bq_sb = loadsm("bq_sb", bq)
            bk_sb = loadsm("bk_sb", bk)
            bva_sb = loadsm("bva_sb", bva, BF16)
            ones_sq = const.tile([128, 128], F32R, name="ones_sq", tag="ones_sq")
            nc.vector.memset(_rd(ones_sq), 1.0)
            ones_bf = const.tile([1, 128], BF16, name="ones_bf", tag="ones_bf")
            nc.vector.memset(ones_bf, 1.0)
            maskp_sb = const.tile([128, IT, SJ], BF16, name="maskp_sb", tag="maskp_sb")
            nc.sync.dma_start(out=maskp_sb, in_=maskp.rearrange("(i p) j -> p i j", p=128))
            id_sb = const.tile([128, 128], BF16, name="id_sb", tag="id_sb")
            nc.sync.dma_start(out=id_sb, in_=ident)
            wo_sb = loadw("wo_sb", wo, KD, D)
            w1_sb = loadw("w1_sb", w1, KD, DFF)
            bo_sb = loadsm("bo_sb", bo)
            b1_sb = loadsm("b1_sb", b1)
            b2_sb = loadsm("b2_sb", b2)
            g1_sb = loadsm("g1_sb", g1)
            be1_sb = loadsm("be1_sb", be1)
            g2_sb = loadsm("g2_sb", g2)
            be2_sb = loadsm("be2_sb", be2)
            w2_sb = loadw("w2_sb", w2, KF, D)
            zeros_sb = const.tile([128, SJ], F32, name="zeros_sb", tag="zeros_sb")
            nc.vector.memset(zeros_sb, 0.0)
            eps_sb = const.tile([128, 1], F32, name="eps_sb", tag="eps_sb")
            nc.vector.memset(eps_sb, EPS)
            id8_sb = loadsm("id8_sb", id8)
            selp_sb = loadsm("selp_sb", selp, F32R)

            def layernorm(z, g_sb, be_sb, out_name, out_tile):
                """z: f32r [128, KD, SJ]; writes normalized f32r into out_tile."""
                s1 = psum.tile([128, SJ], F32, name=f"s1_{out_name}", tag="ps")
                s2 = psum.tile([128, SJ], F32, name=f"s2_{out_name}", tag="ps")
                for k in range(KD):
                    sqc = scr.tile([128, SJ], F32R, name=f"sq_{out_name}_{k}",
                                   tag="e", bufs=4)
                    nc.scalar.activation(out=sqc, in_=_rd(z[:, k, :]), func=AF.Square)
                    nc.tensor.matmul(s1, lhsT=ones_sq, rhs=z[:, k, :],
                                     start=(k == 0), stop=(k == KD - 1))
                    nc.tensor.matmul(s2, lhsT=ones_sq, rhs=sqc,
                                     start=(k == 0), stop=(k == KD - 1))
                mean = scr.tile([128, SJ], F32, name=f"mean_{out_name}", tag="mean", bufs=1)
                rstd = scr.tile([128, SJ], F32, name=f"rstd_{out_name}", tag="rstd", bufs=1)
                nc.vector.tensor_scalar(out=mean, in0=s1, scalar1=1.0 / D,
                                        scalar2=None, op0=OP.mult)
                msq = scr.tile([128, SJ], F32, name=f"msq_{out_name}", tag="e", bufs=4)
                nc.vector.tensor_tensor(out=msq, in0=mean, in1=mean, op=OP.mult)
                var = scr.tile([128, SJ], F32, name=f"var_{out_name}", tag="e", bufs=4)
                nc.vector.scalar_tensor_tensor(out=var, in0=s2, scalar=1.0 / D,
                                               in1=msq, op0=OP.mult, op1=OP.subtract)
                nc.scalar.activation(out=var, in_=var, func=AF.Sqrt, bias=eps_sb[:, 0:1])
                nc.vector.reciprocal(out=rstd, in_=var)
                for k in range(KD):
                    t = scr.tile([128, SJ], F32, name=f"t_{out_name}_{k}",
                                 tag="e", bufs=4)
                    nc.vector.tensor_tensor(out=t, in0=_rd(z[:, k, :]), in1=mean,
                                            op=OP.subtract)
                    nc.vector.tensor_tensor(out=t, in0=t, in1=rstd, op=OP.mult)
                    nc.vector.tensor_scalar(out=out_tile[:, k, :], in0=t,
                                            scalar1=g_sb[:, k:k + 1],
                                            scalar2=be_sb[:, k:k + 1],
                                            op0=OP.mult, op1=OP.add)

            # --- phase helpers (closures over the current layer's tiles) ---
            def a_own(ly, qT, kT, v_sb, hb_t):
                """projections that need only the own (local-first) half of h"""
                for m in range(KD):
                    k_ps = psum.tile([128, SJ], F32, name=f"k_ps_{ly}_{m}", tag="ps")
                    for k in range(KD):
                        nc.tensor.matmul(k_ps, lhsT=wk_sb[:, k, m * 128:(m + 1) * 128],
                                         rhs=hb_t[:, k, 0:SJ],
                                         start=(k == 0), stop=(k == KD - 1))
                    nc.scalar.activation(out=kT[:, m, :], in_=k_ps,
                                         func=AF.Identity, bias=bk_sb[:, m:m + 1])
                for m in range(KD):
                    q_ps = psum.tile([128, SJ], F32, name=f"q_ps_{ly}_{m}_0", tag="ps")
                    for k in range(KD):
                        nc.tensor.matmul(q_ps, lhsT=wq_sb[:, k, m * 128:(m + 1) * 128],
                                         rhs=hb_t[:, k, 0:SJ],
                                         start=(k == 0), stop=(k == KD - 1))
                    nc.scalar.activation(out=qT[:, m, 0:SJ], in_=q_ps,
                                         func=AF.Identity, bias=bq_sb[:, m:m + 1])
                for it in range(IT // 2):
                    for hf in range(2):
                        v_ps = psum.tile([128, 260], F32, name=f"v_ps_{ly}_{it}_{hf}",
                                         tag="ps")
                        for k in range(KD):
                            nc.tensor.matmul(v_ps,
                                             lhsT=hb_t[:, k, it * 128:(it + 1) * 128],
                                             rhs=wva_sb[:, k, hf * 260:(hf + 1) * 260],
                                             start=(k == 0), stop=False)
                        nc.tensor.matmul(v_ps, lhsT=ones_bf,
                                         rhs=bva_sb[0:1, hf * 260:(hf + 1) * 260],
                                         start=False, stop=True)
                        nc.vector.tensor_copy(out=v_sb[:, it, hf, :], in_=v_ps)

            def a_peer(ly, qT, v_sb, hb_t):
                """projections needing the peer half of h"""
                for m in range(KD):
                    q_ps = psum.tile([128, SJ], F32, name=f"q_ps_{ly}_{m}_1", tag="ps")
                    for k in range(KD):
                        nc.tensor.matmul(q_ps, lhsT=wq_sb[:, k, m * 128:(m + 1) * 128],
                                         rhs=hb_t[:, k, SJ:S],
                                         start=(k == 0), stop=(k == KD - 1))
                    nc.scalar.activation(out=qT[:, m, SJ:S], in_=q_ps,
                                         func=AF.Identity, bias=bq_sb[:, m:m + 1])
                for it in range(IT // 2, IT):
                    for hf in range(2):
                        v_ps = psum.tile([128, 260], F32, name=f"v_ps_{ly}_{it}_{hf}",
                                         tag="ps")
                        for k in range(KD):
                            nc.tensor.matmul(v_ps,
                                             lhsT=hb_t[:, k, it * 128:(it + 1) * 128],
                                             rhs=wva_sb[:, k, hf * 260:(hf + 1) * 260],
                                             start=(k == 0), stop=False)
                        nc.tensor.matmul(v_ps, lhsT=ones_bf,
                                         rhs=bva_sb[0:1, hf * 260:(hf + 1) * 260],
                                         start=False, stop=True)
                        nc.vector.tensor_copy(out=v_sb[:, it, hf, :], in_=v_ps)

            def attn_pair(ly, hp, qT, kT, v_sb, outT, zparts):
                o_ps = [psum.tile([65, SJ], F32, name=f"o_ps_{ly}_{hp}_{hr}", tag="ps")
                        for hr in range(2)]
                for it in range(IT):
                    l_ps = psum.tile([128, 2, SJ], F32,
                                     name=f"l_ps_{ly}_{hp}_{it}", tag="ps2", bufs=2)
                    for hr in range(2):
                        nc.tensor.matmul(l_ps[:, hr, :], lhsT=id_sb,
                                         rhs=maskp_sb[:, it, :], start=True, stop=False)
                    for hr in range(2):
                        pb = 64 * hr
                        nc.tensor.matmul(l_ps[:, hr, :],
                                         lhsT=qT[pb:pb + 64, hp, it * 128:(it + 1) * 128],
                                         rhs=kT[pb:pb + 64, hp, :],
                                         start=False, stop=True)
                    e = scr.tile([128, 2, SJ], BF16,
                                 name=f"e_{ly}_{hp}_{it}", tag="e", bufs=4)
                    nc.scalar.activation(out=e, in_=l_ps, func=AF.Exp, scale=0.125)
                    for hr in range(2):
                        h_abs = 2 * hp + hr
                        nc.tensor.matmul(
                            o_ps[hr],
                            lhsT=v_sb[:, it, h_abs // 4,
                                      65 * (h_abs % 4):65 * (h_abs % 4) + 65],
                            rhs=e[:, hr, :], start=(it == 0), stop=(it == IT - 1))
                for hr in range(2):
                    h_abs = 2 * hp + hr
                    nc.vector.reduce_sum(out=zparts[64:65, h_abs:h_abs + 1],
                                         in_=o_ps[hr][64:65, :], axis=AX.X)
                    nc.vector.tensor_copy(out=outT[64 * hr:64 * hr + 64, hp, :],
                                            in_=o_ps[hr][0:64, :])

            def zcc_launch(ly, hp, zparts):
                ci = dram.tile([1, 2], F32, name=f"ccz_in_{ly}_{hp}",
                               tag=f"ccz_in_{ly}_{hp}")
                co = dram.tile([1, 2], F32, name=f"ccz_out_{ly}_{hp}",
                               tag=f"ccz_out_{ly}_{hp}")
                nc.sync.dma_start(out=ci, in_=zparts[64:65, 2 * hp:2 * hp + 2])
                nc.gpsimd.collective_compute("AllReduce", OP.add, replica_groups=GROUPS,
                                             ins=[ci.opt()], outs=[co.opt()])
                return co

            def zchain_c(ly, hp, co, outT, ap_ps):
                """1/Z for pair hp, scale outT pair, accumulate its out-proj part"""
                z2c = scr.tile([2, 1], F32, name=f"z2c_{ly}_{hp}", tag="z2c", bufs=2)
                nc.sync.dma_start(out=z2c, in_=bass.AP(tensor=co.tensor,
                                                       offset=co.offset,
                                                       ap=[[1, 2], [1, 1]]))
                z2i = scr.tile([2, 1], F32, name=f"z2i_{ly}_{hp}", tag="z2i", bufs=2)
                nc.vector.reciprocal(out=z2i, in_=z2c)
                dg2 = scr.tile([2, 2], F32R, name=f"dg2_{ly}_{hp}", tag="dg2", bufs=2)
                nc.vector.tensor_scalar(out=dg2, in0=id8_sb[0:2, 0:2], scalar1=z2i,
                                        scalar2=None, op0=OP.mult)
                zps = psum.tile([128, 2], F32, name=f"zps_{ly}_{hp}", tag="ps2", bufs=2)
                nc.tensor.matmul(zps, lhsT=selp_sb[0:2, :], rhs=dg2,
                                 start=True, stop=True)
                zinv = scr.tile([128, 1], F32, name=f"zinv_{ly}_{hp}", tag="zinv", bufs=2)
                nc.vector.reduce_sum(out=zinv, in_=zps, axis=AX.X)
                nc.vector.tensor_scalar(out=outT[:, hp, :], in0=outT[:, hp, :],
                                        scalar1=zinv, scalar2=None, op0=OP.mult)
                for dt_ in range(KD):
                    nc.tensor.matmul(ap_ps[dt_],
                                     lhsT=wo_sb[:, hp, dt_ * 128:(dt_ + 1) * 128],
                                     rhs=outT[:, hp, :],
                                     start=(hp == 0), stop=(hp == KD - 1))

            # ================= main layer loop =================
            pend = None  # deferred boundary work (set at end of layer 0)
            for ly in range(2):
                qT = state.tile([128, KD, S], BF16, name=f"qT_{ly}", tag="qT", bufs=1)
                kT = state.tile([128, KD, SJ], BF16, name=f"kT_{ly}", tag="kT", bufs=1)
                v_sb = state.tile([128, IT, 2, 260], BF16, name=f"v_{ly}", tag="v",
                                  bufs=1)
                if ly == 0:
                    a_own(ly, qT, kT, v_sb, hb)
                    a_peer(ly, qT, v_sb, hb)
                else:
                    # own-half projections can start before the h-exchange lands
                    a_own(ly, qT, kT, v_sb, hb)
                    ccsum = state.tile([128, KD, SJ], F32, name="ccsum_0",
                                       tag="ccs", bufs=1)
                    nc.sync.dma_start(out=ccsum, in_=pend["ccs_out"].rearrange(
                        "(k p) s -> p k s", p=128))
                    h_next, hb_next = pend["h"], pend["hb"]
                    for k in range(KD):
                        nc.vector.tensor_tensor(out=hb_next[:, k, SJ:S],
                                                in0=ccsum[:, k, :],
                                                in1=_rd(h_next[:, k, 0:SJ]),
                                                op=OP.subtract)
                    a_peer(ly, qT, v_sb, hb)

                # ---- B + Z pipeline + C accumulation ----
                outT = state.tile([128, KD, SJ], BF16, name=f"outT_{ly}", tag="outT",
                                  bufs=1)
                zparts = scr.tile([65, 8], F32, name=f"zp_{ly}", tag="zp", bufs=1)
                ap_ps = [psum.tile([128, SJ], F32, name=f"ap_ps_{ly}_{d}", tag="ps")
                         for d in range(KD)]
                cos = []
                for hp in range(KD):
                    attn_pair(ly, hp, qT, kT, v_sb, outT, zparts)
                    cos.append(zcc_launch(ly, hp, zparts))
                for hp in range(KD):
                    zchain_c(ly, hp, cos[hp], outT, ap_ps)

                z1 = state.tile([128, KD, SJ], F32R, name=f"z1_{ly}", tag="qz", bufs=1)
                for dt_ in range(KD):
                    nc.vector.scalar_tensor_tensor(
                        out=z1[:, dt_, :], in0=ap_ps[dt_], scalar=bo_sb[:, dt_:dt_ + 1],
                        in1=_rd(h_cur[:, dt_, 0:SJ]), op0=OP.add, op1=OP.add)

                # ---- D: LN1 ----
                h1 = state.tile([128, KD, SJ], F32R, name=f"h1_{ly}", tag="h1", bufs=1)
                layernorm(z1, g1_sb, be1_sb, f"h1_{ly}", h1)
                h1b = state.tile([128, KD, SJ], BF16, name=f"h1b_{ly}", tag="h1b", bufs=1)
                for k in range(KD):
                    nc.vector.tensor_copy(out=h1b[:, k, :], in_=_rd(h1[:, k, :]))

                # ---- E: FFN (full DFF, own j-half), w2 streamed ----
                z2 = state.tile([128, KD, SJ], F32R, name=f"z2_{ly}", tag="qz", bufs=1)
                g_ps = [psum.tile([128, SJ], F32, name=f"g_ps_{ly}_{d}", tag="ps")
                        for d in range(KD)]
                for ft in range(KF):
                    f_ps = psum.tile([128, SJ], F32, name=f"f_ps_{ly}_{ft}", tag="ps2",
                                     bufs=2)
                    for k in range(KD):
                        nc.tensor.matmul(f_ps, lhsT=w1_sb[:, k, ft * 128:(ft + 1) * 128],
                                         rhs=h1b[:, k, :],
                                         start=(k == 0), stop=(k == KD - 1))
                    fr = scr.tile([128, SJ], BF16, name=f"fr_{ly}_{ft}", tag="fr", bufs=3)
                    nc.vector.scalar_tensor_tensor(out=fr, in0=f_ps,
                                                   scalar=b1_sb[:, ft:ft + 1],
                                                   in1=zeros_sb, op0=OP.add, op1=OP.max)
                    for d in range(KD):
                        nc.tensor.matmul(g_ps[d],
                                         lhsT=w2_sb[:, ft, d * 128:(d + 1) * 128],
                                         rhs=fr, start=(ft == 0), stop=(ft == KF - 1))
                for d in range(KD):
                    nc.vector.scalar_tensor_tensor(
                        out=z2[:, d, :], in0=g_ps[d], scalar=b2_sb[:, d:d + 1],
                        in1=_rd(h1[:, d, :]), op0=OP.add, op1=OP.add)

                # ---- F: LN2 -> exchange halves (or final output) ----
                if ly == 0:
                    h_next = state.tile([128, KD, S], F32R, name=f"h_{ly + 1}",
                                        tag="h", bufs=2)
                    layernorm(z2, g2_sb, be2_sb, f"hs_{ly}", h_next[:, :, 0:SJ])
                    ccs_in = dram.tile([D, SJ], F32, name=f"ccs_in_{ly}",
                                       tag=f"ccs_in_{ly}")
                    ccs_out = dram.tile([D, SJ], F32, name=f"ccs_out_{ly}",
                                        tag=f"ccs_out_{ly}")
                    nc.sync.dma_start(out=ccs_in.rearrange("(k p) s -> p k s", p=128),
                                      in_=_rd(h_next[:, :, 0:SJ]))
                    nc.gpsimd.collective_compute("AllReduce", OP.add,
                                                 replica_groups=GROUPS,
                                                 ins=[ccs_in.opt()], outs=[ccs_out.opt()])
                    hb_next = state.tile([128, KD, S], BF16, name=f"hb_{ly + 1}",
                                         tag="hb", bufs=2)
                    for k in range(KD):
                        nc.vector.tensor_copy(out=hb_next[:, k, 0:SJ],
                                              in_=_rd(h_next[:, k, 0:SJ]))
                    pend = {"ccs_out": ccs_out, "h": h_next, "hb": hb_next}
                    h_cur = h_next
                    hb = hb_next
                else:
                    hstage = state.tile([128, KD, SJ], F32R, name=f"hs_{ly}", tag="ccs",
                                        bufs=1)
                    layernorm(z2, g2_sb, be2_sb, f"hs_{ly}", hstage)
                    nc.sync.dma_start(out=hout.rearrange("(k p) s -> p k s", p=128),
                                      in_=_rd(hstage))

    nc.compile()
    return nc


_CACHE = {}


def _prep_inputs(x, mask, Wq, bq, Wk, bk, Wv, bv, Wo, bo, W1, b1, W2, b2,
                 g1, be1, g2, be2):
    f32 = np.float32
    x = np.asarray(x, f32)
    mask = np.asarray(mask, f32)
    ident = np.eye(128, dtype=ml_dtypes.bfloat16)

    Wv = np.asarray(Wv, f32)
    bv = np.asarray(bv, f32)
    wva = np.zeros((D, VA), f32)
    bva = np.zeros((1, VA), f32)
    for h in range(H):
        wva[:, 65 * h:65 * h + 64] = Wv[:, 64 * h:64 * h + 64]
        bva[0, 65 * h:65 * h + 64] = bv[64 * h:64 * h + 64]
        bva[0, 65 * h + 64] = 1.0

    def pp(v, cols):
        return np.ascontiguousarray(np.asarray(v, f32).reshape(cols, 128).T)

    selp = np.zeros((H, 128), f32)
    for h in range(H):
        selp[h, (h % 2) * 64:(h % 2) * 64 + 64] = 1.0

    bf16 = ml_dtypes.bfloat16
    common = {
        "id8": np.eye(H, dtype=f32),
        "selp": selp,
        "ident": ident,
        "wq": np.asarray(Wq, f32).astype(bf16),
        "wk": np.asarray(Wk, f32).astype(bf16),
        "wva": wva.astype(bf16),
        "wo": np.asarray(Wo, f32).astype(bf16),
        "w1": np.asarray(W1, f32).astype(bf16),
        "w2": np.asarray(W2, f32).astype(bf16),
        "bq": pp(bq, KD),
        "bk": pp(bk, KD),
        "bva": bva.astype(bf16),
        "bo": pp(bo, KD),
        "b1": pp(b1, KF),
        "b2": pp(b2, KD),
        "g1": pp(g1, KD),
        "be1": pp(be1, KD),
        "g2": pp(g2, KD),
        "be2": pp(be2, KD),
    }
    in_maps = []
    for c in range(N_CORES):
        b, r = c // 2, c % 2
        js = slice(r * SJ, (r + 1) * SJ)
        ps = slice((1 - r) * SJ, (2 - r) * SJ)
        # local token order: own half first (both in h columns and mask rows)
        xb = x[b].T
        m = dict(common)
        xtl = np.ascontiguousarray(np.concatenate([xb[:, js], xb[:, ps]], axis=1))
        m["xT"] = xtl
        m["xb"] = xtl.astype(bf16)
        mrows = np.concatenate([mask[b][js], mask[b][ps]], axis=0)
        m["maskp"] = np.ascontiguousarray(mrows[:, js] * NEG).astype(ml_dtypes.bfloat16)
        in_maps.append(m)
    return in_maps


def get_nc():
    if "nc" not in _CACHE:
        _CACHE["nc"] = build()
    return _CACHE["nc"]


def run(in_maps, **kw):
    nc = get_nc()
    return run_bass_kernel_spmd(nc, in_maps, core_ids=list(range(N_CORES)), **kw)


def kernel(**inputs):
    in_maps = _prep_inputs(**inputs)
    res = run(in_maps)
    out = np.empty((B, S, D), np.float32)
    for c in range(N_CORES):
        b, r = c // 2, c % 2
        out[b, r * SJ:(r + 1) * SJ, :] = res.results[c]["hout"].T
    return out
